# revision 17
# baseline (speedup 1.0000x reference)
"""Causal (inverted-mask) multi-head attention on 8 Trainium2 NeuronCores.

Full inputs in, full output out.  Sharding: core = (batch b, head-group g),
b = core % 4, g = core // 4.  Each core computes, for its batch and its 8
heads, the attention output and a partial output projection
y_part = attn_out @ wo[:, g*512:(g+1)*512].T ; the host sums the two
g-partials per batch (the "all-reduce after wo" done on host).

Module semantics reproduced (from the reference):
  q = x @ wq.T ; k = x @ wk.T ; v = x @ wv.T          (per-head dim 64)
  scores = q k^T / sqrt(1024)
  mask: positions STRICTLY ABOVE the diagonal keep their scores, the lower
  triangle incl. diagonal gets -1e9 (inverted causal mask).  Row S-1 is
  fully masked -> softmax is uniform -> out row = mean(v) (host fix-up).
  out = softmax(scores) @ v ; y = out @ wo.T

Key structure (v2, rebuilt around the measured bottlenecks):
  - scores come out the PE transposed (keys on partitions, queries free);
    the two heads of a head-pair run as row-tiled (0,0)/(64,0) matmuls so
    the pair streams concurrently through the full 128-row array.
  - both halves' scores land in ONE two-bank PSUM tile [128, 1024]; a
    single fused exp instruction covers both, halving exp-call overhead.
  - exp is split between the ACT engine (table exp, scale immediate) and a
    custom 8-stage DVE op (monic cubic ^8: exp(y)=e(y/8)^8, no shifts --
    shifts are unimplemented on trn2 DVE).  The score scale 1/sqrt(D) and
    the cubic's substitution constant are folded into wk on the host, so
    both engines read the same PSUM scores.  A greedy cost model assigns
    each tile to whichever engine is less loaded.
  - the PV matmul keeps a ones-column in V (M=65) so softmax denominators
    accumulate in the same matmul; normalization uses the fast approx
    reciprocal custom op + a GpSimd partition broadcast, off the hot
    engines.
  - only non-fully-masked key blocks are computed (~half the attention
    FLOPs); the last 64 rows are recomputed on host in fp64 (cheap).
"""

import os
import sys

for _p in ("/opt/trn_rl_repo",):
    if os.path.isdir(_p) and _p not in sys.path:
        sys.path.insert(0, _p)

import numpy as np
import ml_dtypes

import concourse.bass as bass
import concourse.mybir as mybir
import concourse.tile as tile
from concourse import bacc
from concourse.bass_utils import run_bass_kernel_spmd

F32 = mybir.dt.float32
BF16 = mybir.dt.bfloat16
EXP = mybir.ActivationFunctionType.Exp

B, S, D = 4, 2048, 1024
NH = 16            # total heads
HD = D // NH       # 64
NHL = NH // 2      # heads per core (head-group)
MG = NHL * HD      # 512 = per-core projection dim
KT = D // 128      # 8 contraction k-tiles
MT = MG // 128     # 4 m-tiles (2 heads each)
CH1 = 512          # projection s-chunk
CH2 = 512          # attention query chunk
NEG = -1.0e9
NPDT = ml_dtypes.bfloat16

# ---- custom DVE exp: exp(y) = e(y/8)^8, e = monic-factored minimax cubic.
# e(w) ~ a0+a1 w+a2 w^2+a3 w^3 on w in [-0.25, 0.25]; substituting w = a3^(-1/3) z
# makes the cubic monic:  M(z) = (z+C0) * (z*(z+C1) + C2),  out = M^8.
# The 1/sqrt(D) softmax scale and the substitution are folded into wk on the
# host: psum score z = score / (256*alpha).  ACT tiles then use scale=8*alpha.
_EXP_C0 = 0.8846494767788311
_EXP_C1 = 0.7782704557181177
_EXP_C2 = 1.1303661464026933
_ALPHA = 1.8187808115874455
WK_PRESCALE = 1.0 / (256.0 * _ALPHA)
ACT_SCALE = 8.0 * _ALPHA
USE_DVE_EXP = True


def _exp8_ref(in0, in1, s0, s1, imm2):
    f = np.float32
    z = in0.astype(np.float32)
    w1 = f(z + f(s0))
    v1 = f(z + f(s1))
    v2 = f(z * v1)
    v3 = f(v2 + f(imm2))
    M = f(w1 * v3)
    M2 = f(M * M)
    M4 = f(M2 * M2)
    return f(M4 * M4)


def _get_exp_op():
    import concourse.dve_ops as dvo

    if hasattr(dvo, "EXP8_CUBIC"):
        return dvo.EXP8_CUBIC
    from concourse.dve_spec import Spec, Src0, C0, C1, C2, lower
    from concourse.dve_uop import DveOpSpec

    w1 = Src0 + C0
    v1 = Src0 + C1
    v2 = Src0 * v1
    v3 = v2 + C2
    M = w1 * v3
    M2 = M * M
    M4 = M2 * M2
    spec = Spec(body=M4 * M4, reference=_exp8_ref)
    row = max(dvo._SUB_OPCODE_FOR_NAME.values()) + 1
    shas = {}
    for ver in ("v3", "v4"):
        shas[ver] = DveOpSpec(
            name="EXP8_CUBIC", opcode=row, uops=lower(spec, ver=ver), rd1_en=False
        ).sha(ver)
    op = dvo.DveOp("EXP8_CUBIC", spec, subdim=False, uops_sha=shas)
    dvo.OPS.append(op)
    dvo.CUSTOM_DVE_SPECS[op.name] = op.spec
    dvo._SUB_OPCODE_FOR_NAME[op.name] = row
    dvo.EXP8_CUBIC = op
    return op


def build_program(s=S, trace_sim=False):
    """Build the SPMD single-core program (parameterized seq len for sim)."""
    nch2 = s // CH2
    jb_n = s // 128
    nch1 = s // CH1
    exp_op = _get_exp_op() if USE_DVE_EXP else None

    nc = bacc.Bacc("TRN2", target_bir_lowering=False, debug=False, num_devices=8)

    xt = nc.dram_tensor("xt", [nch1, 128, KT, CH1], BF16, kind="ExternalInput")
    wqT = nc.dram_tensor("wqT", [128, KT, MG], BF16, kind="ExternalInput")
    wkT = nc.dram_tensor("wkT", [128, KT, MG], BF16, kind="ExternalInput")
    wvT = nc.dram_tensor("wvT", [128, KT, MG], BF16, kind="ExternalInput")
    woT = nc.dram_tensor("woT", [128, MT, D], BF16, kind="ExternalInput")
    trim = nc.dram_tensor("trim", [128, 128], BF16, kind="ExternalInput")
    vones = nc.dram_tensor("vones", [128, s // 128, NHL, 1], BF16,
                           kind="ExternalInput")
    y = nc.dram_tensor("y", [s, D], F32, kind="ExternalOutput")

    # greedy engine-balance accumulators (ns) for the attention phase;
    # DVE starts with a handicap for un-modeled queue overheads (measured
    # ~36us imbalance at equal modeled cost)
    eng = {"act": 0.0, "dve": 36000.0}

    def exp_cost_act(nfree):
        return (nfree + 352) / 1.2

    def exp_cost_dve(nfree):
        # measured: custom-DVE ~0.64ns/elem + ~250ns fixed (1224ns @ 1536)
        return nfree * 0.64 + 250.0

    with tile.TileContext(nc, trace_sim=trace_sim) as tc:
        with tc.tile_pool(name="persist", bufs=1) as pp:
            # ---- persistent SBUF ----
            qt = [pp.tile([128, s], BF16, tag=f"qt{m}", name=f"qt{m}")
                  for m in range(MT)]
            kt = [pp.tile([128, s], BF16, tag=f"kt{m}", name=f"kt{m}")
                  for m in range(MT)]
            # V with a ones column per head: [s-part, block, head, hd+1]
            vt = pp.tile([128, jb_n, NHL, HD + 1], BF16, tag="vt", name="vt")
            tri = pp.tile([128, 128], BF16, tag="tri", name="tri")
            # wo is needed only in phase 2 -- load it first on the sync ring,
            # which pays a ~15-19us warmup on its first large DMA
            wo_sb = [pp.tile([128, D], BF16, tag=f"wo{m}", name=f"wo{m}")
                     for m in range(MT)]
            for m in range(MT):
                nc.sync.dma_start(wo_sb[m][:], woT[:, m, :])

            nc.sync.dma_start(tri[:], trim[:])
            # ones column per (block, head) for the softmax denominator
            nc.sync.dma_start(vt[:, :, :, HD:HD + 1], vones[:])

            # =========== phase 1: projections (single x stream) ===========
            with (
                tc.tile_pool(name="ph1", bufs=2) as p1,
                tc.tile_pool(name="wsb", bufs=1) as pw,
                tc.tile_pool(name="ppsum", bufs=2, space="PSUM") as pps,
            ):
                wq_sb = pw.tile([128, KT, MG], BF16, tag="wq", name="wq_sb")
                wk_sb = pw.tile([128, KT, MG], BF16, tag="wk", name="wk_sb")
                wv_sb = pw.tile([128, KT, MG], BF16, tag="wv", name="wv_sb")
                xc0 = p1.tile([128, KT, CH1], BF16, tag="xc", name="xc")
                # phase-1-critical loads go on the ACT hwdge ring, which is
                # warm from the start (the sync ring's first big DMA pays the
                # warmup on the wo load above); k=0..1 slices first so the
                # first matmul group starts early
                nc.scalar.dma_start(wq_sb[:, 0:2, :], wqT[:, 0:2, :])
                nc.scalar.dma_start(xc0[:, 0:2, :], xt[0][:, 0:2, :])
                nc.scalar.dma_start(wq_sb[:, 2:, :], wqT[:, 2:, :])
                nc.scalar.dma_start(xc0[:, 2:, :], xt[0][:, 2:, :])
                nc.scalar.dma_start(wk_sb[:], wkT[:])
                nc.scalar.dma_start(wv_sb[:], wvT[:])

                for c in range(nch1):
                    xc = xc0 if c == 0 else p1.tile(
                        [128, KT, CH1], BF16, tag="xc", name="xc")
                    if c > 0:
                        nc.sync.dma_start(xc[:], xt[c])
                    cs = slice(c * CH1, (c + 1) * CH1)
                    for m in range(MT):
                        ms = slice(m * 128, (m + 1) * 128)
                        qp = pps.tile([128, CH1], F32, tag="p1q", name="qp")
                        kp = pps.tile([128, CH1], F32, tag="p1k", name="kp")
                        for k in range(KT):
                            nc.tensor.matmul(
                                qp[:], wq_sb[:, k, ms], xc[:, k, :],
                                start=(k == 0), stop=(k == KT - 1))
                        for k in range(KT):
                            nc.tensor.matmul(
                                kp[:], wk_sb[:, k, ms], xc[:, k, :],
                                start=(k == 0), stop=(k == KT - 1))
                        nc.scalar.copy(qt[m][:, cs], qp[:])
                        nc.scalar.copy(kt[m][:, cs], kp[:])
                    for st in range(CH1 // 128):
                        sb = c * (CH1 // 128) + st
                        vp = pps.tile([128, MG], F32, tag="p1v", name="vp")
                        for k in range(KT):
                            nc.tensor.matmul(
                                vp[:], xc[:, k, st * 128:(st + 1) * 128],
                                wv_sb[:, k, :],
                                start=(k == 0), stop=(k == KT - 1))
                        nc.vector.tensor_copy(
                            vt[:, sb, :, 0:HD],
                            vp[:].rearrange("p (h d) -> p h d", h=NHL))

            # =========== phase 2: attention + out-projection ===========
            with (
                tc.tile_pool(name="ph2", bufs=6) as p2,
                tc.tile_pool(name="pers2", bufs=1) as pp2,
                tc.tile_pool(name="ao", bufs=3) as pao,
                tc.tile_pool(name="nrm", bufs=4) as pn,
                tc.tile_pool(name="yst", bufs=2) as py,
                tc.tile_pool(name="spsum", bufs=2, space="PSUM") as sps,
                tc.tile_pool(name="pvps0", bufs=1, space="PSUM") as pv0p,
                tc.tile_pool(name="pvps1", bufs=1, space="PSUM") as pv1p,
                tc.tile_pool(name="ypsum", bufs=2, space="PSUM") as yps,
            ):
                ys_map = {}

                def outproj_group(cc, st, nn):
                    """One 512-wide psum group of the out-projection for
                    s-tile st of chunk cc.  Split so the groups can be
                    emitted interleaved into the next chunk's jb loop,
                    filling PE bubbles in the score->exp->PV chain."""
                    aocc = aoc_by_chunk[cc]
                    if nn == 0:
                        ys_map[(cc, st)] = py.tile([128, D], F32, tag="ys",
                                                   name="ys")
                    ys = ys_map[(cc, st)]
                    yp = yps.tile([128, 512], F32, tag="yp", name="yp")
                    for m in range(MT):
                        nc.tensor.matmul(
                            yp[:],
                            aocc[m][:, st * 128:(st + 1) * 128],
                            wo_sb[m][:, nn * 512:(nn + 1) * 512],
                            start=(m == 0), stop=(m == MT - 1))
                    # psum -> sbuf evacuation on the less-loaded engine
                    if eng["act"] + 720 <= eng["dve"] + 427:
                        eng["act"] += 720
                        nc.scalar.copy(ys[:, nn * 512:(nn + 1) * 512], yp[:])
                    else:
                        eng["dve"] += 427
                        nc.vector.tensor_copy(
                            ys[:, nn * 512:(nn + 1) * 512], yp[:])
                    if nn == D // 512 - 1:
                        r0 = cc * CH2 + st * 128
                        nc.sync.dma_start(y[r0:r0 + 128, :], ys[:])

                def outproj_stile(cc, st):
                    for nn in range(D // 512):
                        outproj_group(cc, st, nn)

                aoc_by_chunk = {}
                for c in range(nch2):
                    i0 = c * CH2
                    first_jb = 4 * c
                    # diag blocks mid-iteration: the leading full blocks
                    # give the exp engines a head start, the trailing ones
                    # let them catch up; psum start flag is on the first
                    # full-width matmul
                    full = list(range(first_jb + 4, jb_n))
                    diag = [first_jb + 3, first_jb + 2, first_jb + 1, first_jb]
                    mid = (len(full) + 1) // 2
                    jb_order = full[:mid] + diag + full[mid:]
                    # unnormalized attention output per chunk
                    aoc = [pao.tile([128, CH2], BF16, tag=f"ao{m}",
                                    name=f"ao{m}") for m in range(MT)]
                    aoc_by_chunk[c] = aoc
                    for hp in range(MT):  # head pair = m-tile
                        pvA = pv0p.tile([HD + 1, CH2], F32, tag="pv0",
                                        name="pv0")
                        pvB = pv1p.tile([HD + 1, CH2], F32, tag="pv1",
                                        name="pv1")
                        # previous chunk's out-projection groups, emitted
                        # interleaved into this jb loop as PE bubble filler
                        op_at = {}
                        if c > 0:
                            nj = len(jb_order)
                            op_at[max(nj // 3, 1)] = (c - 1, hp, 0)
                            op_at[max(2 * nj // 3, 2)] = (c - 1, hp, 1)
                        for ji, jb in enumerate(jb_order):
                            if ji in op_at:
                                outproj_group(*op_at[ji])
                            rr = jb - first_jb
                            n_r = 128 * (rr + 1) if rr < 4 else CH2
                            ks = slice(jb * 128, (jb + 1) * 128)
                            # both heads' scores -> one 2-bank psum tile
                            sc = sps.tile([128, 2 * CH2], F32, tag="sc",
                                          name="sc")
                            nc.tensor.matmul(
                                sc[:, 0:n_r],
                                kt[hp][0:HD, ks], qt[hp][0:HD, i0:i0 + n_r],
                                start=True, stop=True)
                            nc.tensor.matmul(
                                sc[:, CH2:CH2 + n_r],
                                kt[hp][HD:128, ks], qt[hp][HD:128, i0:i0 + n_r],
                                start=True, stop=True)
                            # one fused exp over both halves (covers the
                            # [n_r, CH2) garbage gap on diag blocks; cheap)
                            et = p2.tile([128, 2 * CH2], BF16, tag="et",
                                         name="et")
                            nfree = CH2 + n_r
                            ca = exp_cost_act(nfree)
                            cd = exp_cost_dve(nfree)
                            if exp_op is None or (eng["act"] + ca
                                                  <= eng["dve"] + cd):
                                eng["act"] += ca
                                nc.scalar.activation(
                                    et[:, 0:nfree], sc[:, 0:nfree], EXP,
                                    scale=ACT_SCALE)
                            else:
                                eng["dve"] += cd
                                nc.vector._custom_dve(
                                    exp_op, out=et[:, 0:nfree],
                                    in0=sc[:, 0:nfree],
                                    s0=_EXP_C0, s1=_EXP_C1, imm2=_EXP_C2)
                            if rr < 4:
                                dg = slice(128 * rr, n_r)
                                dgB = slice(CH2 + 128 * rr, CH2 + n_r)
                                eng["dve"] += 306
                                nc.vector.tensor_mul(et[:, dg], et[:, dg],
                                                     tri[:])
                                nc.vector.tensor_mul(et[:, dgB], et[:, dgB],
                                                     tri[:])
                            st_flag = (jb == jb_order[0])
                            sp_flag = (jb == jb_order[-1])
                            nc.tensor.matmul(
                                pvA[:, 0:n_r], vt[:, jb, 2 * hp, :],
                                et[:, 0:n_r], start=st_flag, stop=sp_flag)
                            nc.tensor.matmul(
                                pvB[:, 0:n_r], vt[:, jb, 2 * hp + 1, :],
                                et[:, CH2:CH2 + n_r],
                                start=st_flag, stop=sp_flag)
                        # normalize: approx-reciprocal of the ones-column
                        # denominators, partition-broadcast on GpSimd, one
                        # fused psum-read multiply per head
                        for half, pv in ((0, pvA), (1, pvB)):
                            # free the pv psum bank ASAP (one staged copy on
                            # each engine in parallel) so the next head-pair's
                            # PV matmuls don't stall on the normalize chain;
                            # den is staged separately because custom DVE ops
                            # require partition-0-based input APs
                            c65 = pn.tile([HD, CH2], F32, tag=f"c65{half}",
                                          name=f"c65{half}")
                            den = pn.tile([1, CH2], F32, tag=f"den{half}",
                                          name=f"den{half}")
                            eng["act"] += 737
                            nc.scalar.copy(c65[:], pv[0:HD, :])
                            eng["dve"] += 430
                            nc.vector.tensor_copy(den[:], pv[HD:HD + 1, :])
                            rcp = pn.tile([1, CH2], F32, tag=f"rcp{half}",
                                          name=f"rcp{half}")
                            eng["dve"] += 690
                            nc.vector.reciprocal_approx_fast(rcp[:], den[:])
                            rcb = pn.tile([HD, CH2], F32, tag=f"rcb{half}",
                                          name=f"rcb{half}")
                            nc.gpsimd.partition_broadcast(rcb[:], rcp[:])
                            eng["dve"] += 427
                            nc.vector.tensor_mul(
                                aoc[hp][half * HD:(half + 1) * HD, :],
                                c65[:], rcb[:])

                # last chunk's out-projection
                for st in range(CH2 // 128):
                    outproj_stile(nch2 - 1, st)

    nc.compile()
    return nc


_CACHE = {}


def _get_program():
    if "nc" not in _CACHE:
        _CACHE["nc"] = build_program()
    return _CACHE["nc"]


def _prep_inputs(x, wq, wk, wv, wo):
    """Per-core input maps. core = b + 4*g."""
    # triangular mask for the 128-col transition block of a diagonal key
    # block: keep (mul by 1) where key jj > query ii, else 0
    trim = np.where(
        np.arange(128)[:, None] > np.arange(128)[None, :], 1.0, 0.0
    ).astype(NPDT)
    vones = np.ones((128, S // 128, NHL, 1), dtype=NPDT)
    wqt = np.ascontiguousarray(wq.T)                      # (D, D): [d, m]
    wkt = np.ascontiguousarray(wk.T) * np.float32(WK_PRESCALE)
    wvt = np.ascontiguousarray(wv.T)
    wot = np.ascontiguousarray(wo.T)                      # [m, n]
    in_maps = []
    xts = {}
    for b in range(B):
        xT = x[b].T.astype(NPDT)  # (D, S)
        # [c, p, k, j] = xT[k*128+p, c*CH1+j]
        xts[b] = np.ascontiguousarray(
            xT.reshape(KT, 128, S // CH1, CH1).transpose(2, 1, 0, 3))

    def wslice(wt, ms):
        # [128, KT, MG] with [p, k, m] = wt[k*128+p, ms][m]
        return np.ascontiguousarray(
            wt[:, ms].reshape(KT, 128, MG).transpose(1, 0, 2).astype(NPDT))

    for core in range(8):
        b, g = core % 4, core // 4
        ms = slice(g * MG, (g + 1) * MG)
        # woT [128, MT, D]: [p, m, n] = wot[g*MG + m*128 + p, n]
        wo_core = np.ascontiguousarray(
            wot[ms, :].reshape(MT, 128, D).transpose(1, 0, 2).astype(NPDT))
        in_maps.append({
            "xt": xts[b],
            "wqT": wslice(wqt, ms),
            "wkT": wslice(wkt, ms),
            "wvT": wslice(wvt, ms),
            "woT": wo_core,
            "trim": trim,
            "vones": vones,
        })
    return in_maps


def _fix_last_rows(out, x, wq, wk, wv, wo, tail=64):
    """The last `tail` rows attend over few keys (no averaging to damp
    device bf16 noise), and row S-1 is fully masked (uniform softmax over
    all S keys).  Recompute them on host in fp64 -- cheap and exact."""
    q0 = S - tail
    wq64, wk64 = wq.astype(np.float64).T, wk.astype(np.float64).T
    wv64, wo64 = wv.astype(np.float64).T, wo.astype(np.float64).T
    hd = D // NH
    for b in range(B):
        xb = x[b].astype(np.float64)
        # row S-1: all keys masked -> uniform attention over all S keys
        vmean = xb.mean(axis=0) @ wv64
        out[b, S - 1, :] = (vmean @ wo64).astype(np.float32)
        # rows q0..S-2: keys strictly after the query, all within [q0+1, S)
        q6 = (xb[q0:S - 1] @ wq64).reshape(tail - 1, NH, hd)
        k6 = (xb[q0 + 1:] @ wk64).reshape(tail - 1, NH, hd)
        v6 = (xb[q0 + 1:] @ wv64).reshape(tail - 1, NH, hd)
        # scores[i, h, j] over keys global (q0+1+j); keep iff j >= i
        sc = np.einsum("ihd,jhd->hij", q6, k6) / np.sqrt(np.float64(D))
        keep = (np.arange(tail - 1)[None, :] >=
                np.arange(tail - 1)[:, None])[None, :, :]
        e = np.where(keep, np.exp(sc - sc.max(axis=2, keepdims=True)), 0.0)
        attn = e / e.sum(axis=2, keepdims=True)
        ao = np.einsum("hij,jhd->ihd", attn, v6).reshape(tail - 1, D)
        out[b, q0:S - 1, :] = (ao @ wo64).astype(np.float32)
    return out


def kernel(x, wq, wk, wv, wo, n_heads=NH, _trace=False):
    x = np.asarray(x, dtype=np.float32)
    wq = np.asarray(wq, dtype=np.float32)
    wk = np.asarray(wk, dtype=np.float32)
    wv = np.asarray(wv, dtype=np.float32)
    wo = np.asarray(wo, dtype=np.float32)

    nc = _get_program()
    in_maps = _prep_inputs(x, wq, wk, wv, wo)
    res = run_bass_kernel_spmd(nc, in_maps, list(range(8)), trace=_trace)
    out = np.zeros((B, S, D), dtype=np.float32)
    for b in range(B):
        out[b] = res.results[b]["y"] + res.results[b + 4]["y"]
    out = _fix_last_rows(out, x, wq, wk, wv, wo)
    if _trace:
        _CACHE["last_results"] = res
    return out


# revision 20
# speedup vs baseline: 1.0394x; 1.0394x over previous
"""Causal (inverted-mask) multi-head attention on 8 Trainium2 NeuronCores.

Full inputs in, full output out.  Sharding: core = (batch b, head-group g),
b = core % 4, g = core // 4.  Each core computes, for its batch and its 8
heads, the attention output and a partial output projection
y_part = attn_out @ wo[:, g*512:(g+1)*512].T ; the host sums the two
g-partials per batch (the "all-reduce after wo" done on host).

Module semantics reproduced (from the reference):
  q = x @ wq.T ; k = x @ wk.T ; v = x @ wv.T          (per-head dim 64)
  scores = q k^T / sqrt(1024)
  mask: positions STRICTLY ABOVE the diagonal keep their scores, the lower
  triangle incl. diagonal gets -1e9 (inverted causal mask).  Row S-1 is
  fully masked -> softmax is uniform -> out row = mean(v) (host fix-up).
  out = softmax(scores) @ v ; y = out @ wo.T

Key structure (v2, rebuilt around the measured bottlenecks):
  - scores come out the PE transposed (keys on partitions, queries free);
    the two heads of a head-pair run as row-tiled (0,0)/(64,0) matmuls so
    the pair streams concurrently through the full 128-row array.
  - both halves' scores land in ONE two-bank PSUM tile [128, 1024]; a
    single fused exp instruction covers both, halving exp-call overhead.
  - exp is split between the ACT engine (table exp, scale immediate) and a
    custom 8-stage DVE op (monic cubic ^8: exp(y)=e(y/8)^8, no shifts --
    shifts are unimplemented on trn2 DVE).  The score scale 1/sqrt(D) and
    the cubic's substitution constant are folded into wk on the host, so
    both engines read the same PSUM scores.  A greedy cost model assigns
    each tile to whichever engine is less loaded.
  - the PV matmul keeps a ones-column in V (M=65) so softmax denominators
    accumulate in the same matmul; normalization uses the fast approx
    reciprocal custom op + a GpSimd partition broadcast, off the hot
    engines.
  - only non-fully-masked key blocks are computed (~half the attention
    FLOPs); the last 64 rows are recomputed on host in fp64 (cheap).
"""

import os
import sys

for _p in ("/opt/trn_rl_repo",):
    if os.path.isdir(_p) and _p not in sys.path:
        sys.path.insert(0, _p)

import numpy as np
import ml_dtypes

import concourse.bass as bass
import concourse.mybir as mybir
import concourse.tile as tile
from concourse import bacc
from concourse.bass_utils import run_bass_kernel_spmd

F32 = mybir.dt.float32
BF16 = mybir.dt.bfloat16
EXP = mybir.ActivationFunctionType.Exp

B, S, D = 4, 2048, 1024
NH = 16            # total heads
HD = D // NH       # 64
NHL = NH // 2      # heads per core (head-group)
MG = NHL * HD      # 512 = per-core projection dim
KT = D // 128      # 8 contraction k-tiles
MT = MG // 128     # 4 m-tiles (2 heads each)
CH1 = 512          # projection s-chunk
CH2 = 512          # attention query chunk
NEG = -1.0e9
NPDT = ml_dtypes.bfloat16

# ---- custom DVE exp: exp(y) = e(y/8)^8, e = monic-factored minimax cubic.
# e(w) ~ a0+a1 w+a2 w^2+a3 w^3 on w in [-0.25, 0.25]; substituting w = a3^(-1/3) z
# makes the cubic monic:  M(z) = (z+C0) * (z*(z+C1) + C2),  out = M^8.
# The 1/sqrt(D) softmax scale and the substitution are folded into wk on the
# host: psum score z = score / (256*alpha).  ACT tiles then use scale=8*alpha.
_EXP_C0 = 0.8846494767788311
_EXP_C1 = 0.7782704557181177
_EXP_C2 = 1.1303661464026933
_ALPHA = 1.8187808115874455
WK_PRESCALE = 1.0 / (256.0 * _ALPHA)
ACT_SCALE = 8.0 * _ALPHA
USE_DVE_EXP = True


def _exp8_ref(in0, in1, s0, s1, imm2):
    f = np.float32
    z = in0.astype(np.float32)
    w1 = f(z + f(s0))
    v1 = f(z + f(s1))
    v2 = f(z * v1)
    v3 = f(v2 + f(imm2))
    M = f(w1 * v3)
    M2 = f(M * M)
    M4 = f(M2 * M2)
    return f(M4 * M4)


def _get_exp_op():
    import concourse.dve_ops as dvo

    if hasattr(dvo, "EXP8_CUBIC"):
        return dvo.EXP8_CUBIC
    from concourse.dve_spec import Spec, Src0, C0, C1, C2, lower
    from concourse.dve_uop import DveOpSpec

    w1 = Src0 + C0
    v1 = Src0 + C1
    v2 = Src0 * v1
    v3 = v2 + C2
    M = w1 * v3
    M2 = M * M
    M4 = M2 * M2
    spec = Spec(body=M4 * M4, reference=_exp8_ref)
    row = max(dvo._SUB_OPCODE_FOR_NAME.values()) + 1
    shas = {}
    for ver in ("v3", "v4"):
        shas[ver] = DveOpSpec(
            name="EXP8_CUBIC", opcode=row, uops=lower(spec, ver=ver), rd1_en=False
        ).sha(ver)
    op = dvo.DveOp("EXP8_CUBIC", spec, subdim=False, uops_sha=shas)
    dvo.OPS.append(op)
    dvo.CUSTOM_DVE_SPECS[op.name] = op.spec
    dvo._SUB_OPCODE_FOR_NAME[op.name] = row
    dvo.EXP8_CUBIC = op
    return op


def build_program(s=S, trace_sim=False):
    """Build the SPMD single-core program (parameterized seq len for sim)."""
    nch2 = s // CH2
    jb_n = s // 128
    nch1 = s // CH1
    exp_op = _get_exp_op() if USE_DVE_EXP else None

    nc = bacc.Bacc("TRN2", target_bir_lowering=False, debug=False, num_devices=8)

    xt = nc.dram_tensor("xt", [nch1, 128, KT, CH1], BF16, kind="ExternalInput")
    wqT = nc.dram_tensor("wqT", [128, KT, MG], BF16, kind="ExternalInput")
    wkT = nc.dram_tensor("wkT", [128, KT, MG], BF16, kind="ExternalInput")
    wvT = nc.dram_tensor("wvT", [128, KT, MG], BF16, kind="ExternalInput")
    woT = nc.dram_tensor("woT", [128, MT, D], BF16, kind="ExternalInput")
    trim = nc.dram_tensor("trim", [128, 128], BF16, kind="ExternalInput")
    vones = nc.dram_tensor("vones", [128, s // 128, NHL, 1], BF16,
                           kind="ExternalInput")
    y = nc.dram_tensor("y", [s, D], F32, kind="ExternalOutput")

    # greedy engine-balance accumulators (ns) for the attention phase;
    # DVE starts with a handicap for un-modeled queue overheads (measured
    # ~36us imbalance at equal modeled cost)
    eng = {"act": 0.0, "dve": 36000.0}

    def exp_cost_act(nfree):
        return (nfree + 352) / 1.2

    def exp_cost_dve(nfree):
        # measured: custom-DVE ~0.64ns/elem + ~250ns fixed (1224ns @ 1536)
        return nfree * 0.64 + 250.0

    with tile.TileContext(nc, trace_sim=trace_sim) as tc:
        with tc.tile_pool(name="persist", bufs=1) as pp:
            # ---- persistent SBUF ----
            qt = [pp.tile([128, s], BF16, tag=f"qt{m}", name=f"qt{m}")
                  for m in range(MT)]
            kt = [pp.tile([128, s], BF16, tag=f"kt{m}", name=f"kt{m}")
                  for m in range(MT)]
            # V with a ones column per head: [s-part, block, head, hd+1]
            vt = pp.tile([128, jb_n, NHL, HD + 1], BF16, tag="vt", name="vt")
            tri = pp.tile([128, 128], BF16, tag="tri", name="tri")
            # wo is needed only in phase 2 -- load it first on the sync ring,
            # which pays a ~15-19us warmup on its first large DMA
            wo_sb = [pp.tile([128, D], BF16, tag=f"wo{m}", name=f"wo{m}")
                     for m in range(MT)]
            for m in range(MT):
                nc.sync.dma_start(wo_sb[m][:], woT[:, m, :])

            nc.sync.dma_start(tri[:], trim[:])
            # ones column per (block, head) for the softmax denominator
            nc.sync.dma_start(vt[:, :, :, HD:HD + 1], vones[:])

            # =========== phase 1: projections (single x stream) ===========
            with (
                tc.tile_pool(name="ph1", bufs=2) as p1,
                tc.tile_pool(name="wsb", bufs=1) as pw,
                tc.tile_pool(name="ppsum", bufs=2, space="PSUM") as pps,
            ):
                wq_sb = pw.tile([128, KT, MG], BF16, tag="wq", name="wq_sb")
                wk_sb = pw.tile([128, KT, MG], BF16, tag="wk", name="wk_sb")
                wv_sb = pw.tile([128, KT, MG], BF16, tag="wv", name="wv_sb")
                xc0 = p1.tile([128, KT, CH1], BF16, tag="xc", name="xc")
                # phase-1-critical loads go on the ACT hwdge ring, which is
                # warm from the start (the sync ring's first big DMA pays the
                # warmup on the wo load above); k=0..1 slices first so the
                # first matmul group starts early
                nc.scalar.dma_start(wq_sb[:, 0:2, :], wqT[:, 0:2, :])
                nc.scalar.dma_start(xc0[:, 0:2, :], xt[0][:, 0:2, :])
                nc.scalar.dma_start(wq_sb[:, 2:, :], wqT[:, 2:, :])
                nc.scalar.dma_start(xc0[:, 2:, :], xt[0][:, 2:, :])
                nc.scalar.dma_start(wk_sb[:], wkT[:])
                nc.scalar.dma_start(wv_sb[:], wvT[:])

                for c in range(nch1):
                    xc = xc0 if c == 0 else p1.tile(
                        [128, KT, CH1], BF16, tag="xc", name="xc")
                    if c > 0:
                        nc.sync.dma_start(xc[:], xt[c])
                    cs = slice(c * CH1, (c + 1) * CH1)
                    for m in range(MT):
                        ms = slice(m * 128, (m + 1) * 128)
                        qp = pps.tile([128, CH1], F32, tag="p1q", name="qp")
                        kp = pps.tile([128, CH1], F32, tag="p1k", name="kp")
                        for k in range(KT):
                            nc.tensor.matmul(
                                qp[:], wq_sb[:, k, ms], xc[:, k, :],
                                start=(k == 0), stop=(k == KT - 1))
                        for k in range(KT):
                            nc.tensor.matmul(
                                kp[:], wk_sb[:, k, ms], xc[:, k, :],
                                start=(k == 0), stop=(k == KT - 1))
                        nc.scalar.copy(qt[m][:, cs], qp[:])
                        nc.scalar.copy(kt[m][:, cs], kp[:])
                    for st in range(CH1 // 128):
                        sb = c * (CH1 // 128) + st
                        vp = pps.tile([128, MG], F32, tag="p1v", name="vp")
                        for k in range(KT):
                            nc.tensor.matmul(
                                vp[:], xc[:, k, st * 128:(st + 1) * 128],
                                wv_sb[:, k, :],
                                start=(k == 0), stop=(k == KT - 1))
                        nc.vector.tensor_copy(
                            vt[:, sb, :, 0:HD],
                            vp[:].rearrange("p (h d) -> p h d", h=NHL))

            # =========== phase 2: attention + out-projection ===========
            with (
                tc.tile_pool(name="ph2", bufs=6) as p2,
                tc.tile_pool(name="pers2", bufs=1) as pp2,
                tc.tile_pool(name="ao", bufs=3) as pao,
                tc.tile_pool(name="nrm", bufs=4) as pn,
                tc.tile_pool(name="yst", bufs=2) as py,
                tc.tile_pool(name="spsum", bufs=5, space="PSUM") as sps,
                tc.tile_pool(name="pvps0", bufs=1, space="PSUM") as pv0p,
                tc.tile_pool(name="pvps1", bufs=1, space="PSUM") as pv1p,
                tc.tile_pool(name="ypsum", bufs=1, space="PSUM") as yps,
            ):
                ys_map = {}

                def outproj_group(cc, st, nn):
                    """One 512-wide psum group of the out-projection for
                    s-tile st of chunk cc.  Split so the groups can be
                    emitted interleaved into the next chunk's jb loop,
                    filling PE bubbles in the score->exp->PV chain."""
                    aocc = aoc_by_chunk[cc]
                    if nn == 0:
                        ys_map[(cc, st)] = py.tile([128, D], F32, tag="ys",
                                                   name="ys")
                    ys = ys_map[(cc, st)]
                    yp = yps.tile([128, 512], F32, tag="yp", name="yp")
                    for m in range(MT):
                        nc.tensor.matmul(
                            yp[:],
                            aocc[m][:, st * 128:(st + 1) * 128],
                            wo_sb[m][:, nn * 512:(nn + 1) * 512],
                            start=(m == 0), stop=(m == MT - 1))
                    # psum -> sbuf evacuation on the less-loaded engine
                    if eng["act"] + 720 <= eng["dve"] + 427:
                        eng["act"] += 720
                        nc.scalar.copy(ys[:, nn * 512:(nn + 1) * 512], yp[:])
                    else:
                        eng["dve"] += 427
                        nc.vector.tensor_copy(
                            ys[:, nn * 512:(nn + 1) * 512], yp[:])
                    if nn == D // 512 - 1:
                        r0 = cc * CH2 + st * 128
                        nc.sync.dma_start(y[r0:r0 + 128, :], ys[:])

                def outproj_stile(cc, st):
                    for nn in range(D // 512):
                        outproj_group(cc, st, nn)

                aoc_by_chunk = {}
                for c in range(nch2):
                    i0 = c * CH2
                    first_jb = 4 * c
                    # diag blocks mid-iteration: the leading full blocks
                    # give the exp engines a head start, the trailing ones
                    # let them catch up; psum start flag is on the first
                    # full-width matmul
                    full = list(range(first_jb + 4, jb_n))
                    diag = [first_jb + 3, first_jb + 2, first_jb + 1, first_jb]
                    mid = (len(full) + 1) // 2
                    jb_order = full[:mid] + diag + full[mid:]
                    # unnormalized attention output per chunk
                    aoc = [pao.tile([128, CH2], BF16, tag=f"ao{m}",
                                    name=f"ao{m}") for m in range(MT)]
                    aoc_by_chunk[c] = aoc
                    for hp in range(MT):  # head pair = m-tile
                        pvA = pv0p.tile([HD + 1, CH2], F32, tag="pv0",
                                        name="pv0")
                        pvB = pv1p.tile([HD + 1, CH2], F32, tag="pv1",
                                        name="pv1")
                        # previous chunk's out-projection groups, emitted
                        # interleaved into this jb loop as PE bubble filler
                        op_at = {}
                        if c > 0:
                            nj = len(jb_order)
                            op_at[max(nj // 3, 1)] = (c - 1, hp, 0)
                            op_at[max(2 * nj // 3, 2)] = (c - 1, hp, 1)
                        for ji, jb in enumerate(jb_order):
                            if ji in op_at:
                                outproj_group(*op_at[ji])
                            rr = jb - first_jb
                            n_r = 128 * (rr + 1) if rr < 4 else CH2
                            ks = slice(jb * 128, (jb + 1) * 128)
                            # per-half single-bank score tiles from one
                            # 5-deep ring: deeper score->exp->free pipeline
                            scA = sps.tile([128, CH2], F32, tag="sc",
                                           name="scA")
                            scB = sps.tile([128, CH2], F32, tag="sc",
                                           name="scB")
                            nc.tensor.matmul(
                                scA[:, 0:n_r],
                                kt[hp][0:HD, ks], qt[hp][0:HD, i0:i0 + n_r],
                                start=True, stop=True)
                            nc.tensor.matmul(
                                scB[:, 0:n_r],
                                kt[hp][HD:128, ks], qt[hp][HD:128, i0:i0 + n_r],
                                start=True, stop=True)
                            # the two halves' exps go to different engines
                            # and run concurrently
                            et = p2.tile([128, 2 * CH2], BF16, tag="et",
                                         name="et")
                            for half, sct in ((0, scA), (1, scB)):
                                dst = et[:, half * CH2:half * CH2 + n_r]
                                ca = exp_cost_act(n_r)
                                cd = exp_cost_dve(n_r)
                                if exp_op is None or (eng["act"] + ca
                                                      <= eng["dve"] + cd):
                                    eng["act"] += ca
                                    nc.scalar.activation(
                                        dst, sct[:, 0:n_r], EXP,
                                        scale=ACT_SCALE)
                                else:
                                    eng["dve"] += cd
                                    nc.vector._custom_dve(
                                        exp_op, out=dst, in0=sct[:, 0:n_r],
                                        s0=_EXP_C0, s1=_EXP_C1, imm2=_EXP_C2)
                            if rr < 4:
                                dg = slice(128 * rr, n_r)
                                dgB = slice(CH2 + 128 * rr, CH2 + n_r)
                                eng["dve"] += 306
                                nc.vector.tensor_mul(et[:, dg], et[:, dg],
                                                     tri[:])
                                nc.vector.tensor_mul(et[:, dgB], et[:, dgB],
                                                     tri[:])
                            st_flag = (jb == jb_order[0])
                            sp_flag = (jb == jb_order[-1])
                            nc.tensor.matmul(
                                pvA[:, 0:n_r], vt[:, jb, 2 * hp, :],
                                et[:, 0:n_r], start=st_flag, stop=sp_flag)
                            nc.tensor.matmul(
                                pvB[:, 0:n_r], vt[:, jb, 2 * hp + 1, :],
                                et[:, CH2:CH2 + n_r],
                                start=st_flag, stop=sp_flag)
                        # normalize: approx-reciprocal of the ones-column
                        # denominators, partition-broadcast on GpSimd, one
                        # fused psum-read multiply per head
                        for half, pv in ((0, pvA), (1, pvB)):
                            # custom DVE ops require partition-0-based input
                            # APs: stage the denominator row first
                            den = pn.tile([1, CH2], F32, tag=f"den{half}",
                                          name=f"den{half}")
                            eng["act"] += 720
                            nc.scalar.copy(den[:], pv[HD:HD + 1, :])
                            rcp = pn.tile([1, CH2], F32, tag=f"rcp{half}",
                                          name=f"rcp{half}")
                            eng["dve"] += 690
                            nc.vector.reciprocal_approx_fast(rcp[:], den[:])
                            rcb = pn.tile([HD, CH2], F32, tag=f"rcb{half}",
                                          name=f"rcb{half}")
                            nc.gpsimd.partition_broadcast(rcb[:], rcp[:])
                            eng["dve"] += 427
                            nc.vector.tensor_mul(
                                aoc[hp][half * HD:(half + 1) * HD, :],
                                pv[0:HD, :], rcb[:])

                # last chunk's out-projection
                for st in range(CH2 // 128):
                    outproj_stile(nch2 - 1, st)

    nc.compile()
    return nc


_CACHE = {}


def _get_program():
    if "nc" not in _CACHE:
        _CACHE["nc"] = build_program()
    return _CACHE["nc"]


def _prep_inputs(x, wq, wk, wv, wo):
    """Per-core input maps. core = b + 4*g."""
    # triangular mask for the 128-col transition block of a diagonal key
    # block: keep (mul by 1) where key jj > query ii, else 0
    trim = np.where(
        np.arange(128)[:, None] > np.arange(128)[None, :], 1.0, 0.0
    ).astype(NPDT)
    vones = np.ones((128, S // 128, NHL, 1), dtype=NPDT)
    wqt = np.ascontiguousarray(wq.T)                      # (D, D): [d, m]
    wkt = np.ascontiguousarray(wk.T) * np.float32(WK_PRESCALE)
    wvt = np.ascontiguousarray(wv.T)
    wot = np.ascontiguousarray(wo.T)                      # [m, n]
    in_maps = []
    xts = {}
    for b in range(B):
        xT = x[b].T.astype(NPDT)  # (D, S)
        # [c, p, k, j] = xT[k*128+p, c*CH1+j]
        xts[b] = np.ascontiguousarray(
            xT.reshape(KT, 128, S // CH1, CH1).transpose(2, 1, 0, 3))

    def wslice(wt, ms):
        # [128, KT, MG] with [p, k, m] = wt[k*128+p, ms][m]
        return np.ascontiguousarray(
            wt[:, ms].reshape(KT, 128, MG).transpose(1, 0, 2).astype(NPDT))

    for core in range(8):
        b, g = core % 4, core // 4
        ms = slice(g * MG, (g + 1) * MG)
        # woT [128, MT, D]: [p, m, n] = wot[g*MG + m*128 + p, n]
        wo_core = np.ascontiguousarray(
            wot[ms, :].reshape(MT, 128, D).transpose(1, 0, 2).astype(NPDT))
        in_maps.append({
            "xt": xts[b],
            "wqT": wslice(wqt, ms),
            "wkT": wslice(wkt, ms),
            "wvT": wslice(wvt, ms),
            "woT": wo_core,
            "trim": trim,
            "vones": vones,
        })
    return in_maps


def _fix_last_rows(out, x, wq, wk, wv, wo, tail=64):
    """The last `tail` rows attend over few keys (no averaging to damp
    device bf16 noise), and row S-1 is fully masked (uniform softmax over
    all S keys).  Recompute them on host in fp64 -- cheap and exact."""
    q0 = S - tail
    wq64, wk64 = wq.astype(np.float64).T, wk.astype(np.float64).T
    wv64, wo64 = wv.astype(np.float64).T, wo.astype(np.float64).T
    hd = D // NH
    for b in range(B):
        xb = x[b].astype(np.float64)
        # row S-1: all keys masked -> uniform attention over all S keys
        vmean = xb.mean(axis=0) @ wv64
        out[b, S - 1, :] = (vmean @ wo64).astype(np.float32)
        # rows q0..S-2: keys strictly after the query, all within [q0+1, S)
        q6 = (xb[q0:S - 1] @ wq64).reshape(tail - 1, NH, hd)
        k6 = (xb[q0 + 1:] @ wk64).reshape(tail - 1, NH, hd)
        v6 = (xb[q0 + 1:] @ wv64).reshape(tail - 1, NH, hd)
        # scores[i, h, j] over keys global (q0+1+j); keep iff j >= i
        sc = np.einsum("ihd,jhd->hij", q6, k6) / np.sqrt(np.float64(D))
        keep = (np.arange(tail - 1)[None, :] >=
                np.arange(tail - 1)[:, None])[None, :, :]
        e = np.where(keep, np.exp(sc - sc.max(axis=2, keepdims=True)), 0.0)
        attn = e / e.sum(axis=2, keepdims=True)
        ao = np.einsum("hij,jhd->ihd", attn, v6).reshape(tail - 1, D)
        out[b, q0:S - 1, :] = (ao @ wo64).astype(np.float32)
    return out


def kernel(x, wq, wk, wv, wo, n_heads=NH, _trace=False):
    x = np.asarray(x, dtype=np.float32)
    wq = np.asarray(wq, dtype=np.float32)
    wk = np.asarray(wk, dtype=np.float32)
    wv = np.asarray(wv, dtype=np.float32)
    wo = np.asarray(wo, dtype=np.float32)

    nc = _get_program()
    in_maps = _prep_inputs(x, wq, wk, wv, wo)
    res = run_bass_kernel_spmd(nc, in_maps, list(range(8)), trace=_trace)
    out = np.zeros((B, S, D), dtype=np.float32)
    for b in range(B):
        out[b] = res.results[b]["y"] + res.results[b + 4]["y"]
    out = _fix_last_rows(out, x, wq, wk, wv, wo)
    if _trace:
        _CACHE["last_results"] = res
    return out


# revision 22
# speedup vs baseline: 1.0755x; 1.0346x over previous
"""Causal (inverted-mask) multi-head attention on 8 Trainium2 NeuronCores.

Full inputs in, full output out.  Sharding: core = (batch b, head-group g),
b = core % 4, g = core // 4.  Each core computes, for its batch and its 8
heads, the attention output and a partial output projection
y_part = attn_out @ wo[:, g*512:(g+1)*512].T ; the host sums the two
g-partials per batch (the "all-reduce after wo" done on host).

Module semantics reproduced (from the reference):
  q = x @ wq.T ; k = x @ wk.T ; v = x @ wv.T          (per-head dim 64)
  scores = q k^T / sqrt(1024)
  mask: positions STRICTLY ABOVE the diagonal keep their scores, the lower
  triangle incl. diagonal gets -1e9 (inverted causal mask).  Row S-1 is
  fully masked -> softmax is uniform -> out row = mean(v) (host fix-up).
  out = softmax(scores) @ v ; y = out @ wo.T

Key structure (v2, rebuilt around the measured bottlenecks):
  - scores come out the PE transposed (keys on partitions, queries free);
    the two heads of a head-pair run as row-tiled (0,0)/(64,0) matmuls so
    the pair streams concurrently through the full 128-row array.
  - both halves' scores land in ONE two-bank PSUM tile [128, 1024]; a
    single fused exp instruction covers both, halving exp-call overhead.
  - exp is split between the ACT engine (table exp, scale immediate) and a
    custom 8-stage DVE op (monic cubic ^8: exp(y)=e(y/8)^8, no shifts --
    shifts are unimplemented on trn2 DVE).  The score scale 1/sqrt(D) and
    the cubic's substitution constant are folded into wk on the host, so
    both engines read the same PSUM scores.  A greedy cost model assigns
    each tile to whichever engine is less loaded.
  - the PV matmul keeps a ones-column in V (M=65) so softmax denominators
    accumulate in the same matmul; normalization uses the fast approx
    reciprocal custom op + a GpSimd partition broadcast, off the hot
    engines.
  - only non-fully-masked key blocks are computed (~half the attention
    FLOPs); the last 64 rows are recomputed on host in fp64 (cheap).
"""

import os
import sys

for _p in ("/opt/trn_rl_repo",):
    if os.path.isdir(_p) and _p not in sys.path:
        sys.path.insert(0, _p)

import numpy as np
import ml_dtypes

import concourse.bass as bass
import concourse.mybir as mybir
import concourse.tile as tile
from concourse import bacc
from concourse.bass_utils import run_bass_kernel_spmd

F32 = mybir.dt.float32
BF16 = mybir.dt.bfloat16
EXP = mybir.ActivationFunctionType.Exp

B, S, D = 4, 2048, 1024
NH = 16            # total heads
HD = D // NH       # 64
NHL = NH // 2      # heads per core (head-group)
MG = NHL * HD      # 512 = per-core projection dim
KT = D // 128      # 8 contraction k-tiles
MT = MG // 128     # 4 m-tiles (2 heads each)
CH1 = 512          # projection s-chunk
CH2 = 512          # attention query chunk
NEG = -1.0e9
NPDT = ml_dtypes.bfloat16

# ---- custom DVE exp: exp(y) = e(y/8)^8, e = monic-factored minimax cubic.
# e(w) ~ a0+a1 w+a2 w^2+a3 w^3 on w in [-0.25, 0.25]; substituting w = a3^(-1/3) z
# makes the cubic monic:  M(z) = (z+C0) * (z*(z+C1) + C2),  out = M^8.
# The 1/sqrt(D) softmax scale and the substitution are folded into wk on the
# host: psum score z = score / (256*alpha).  ACT tiles then use scale=8*alpha.
_EXP_C0 = 0.8846494767788311
_EXP_C1 = 0.7782704557181177
_EXP_C2 = 1.1303661464026933
_ALPHA = 1.8187808115874455
WK_PRESCALE = 1.0 / (256.0 * _ALPHA)
ACT_SCALE = 8.0 * _ALPHA
USE_DVE_EXP = True


def _exp8_ref(in0, in1, s0, s1, imm2):
    f = np.float32
    z = in0.astype(np.float32)
    w1 = f(z + f(s0))
    v1 = f(z + f(s1))
    v2 = f(z * v1)
    v3 = f(v2 + f(imm2))
    M = f(w1 * v3)
    M2 = f(M * M)
    M4 = f(M2 * M2)
    return f(M4 * M4)


def _get_exp_op():
    import concourse.dve_ops as dvo

    if hasattr(dvo, "EXP8_CUBIC"):
        return dvo.EXP8_CUBIC
    from concourse.dve_spec import Spec, Src0, C0, C1, C2, lower
    from concourse.dve_uop import DveOpSpec

    w1 = Src0 + C0
    v1 = Src0 + C1
    v2 = Src0 * v1
    v3 = v2 + C2
    M = w1 * v3
    M2 = M * M
    M4 = M2 * M2
    spec = Spec(body=M4 * M4, reference=_exp8_ref)
    row = max(dvo._SUB_OPCODE_FOR_NAME.values()) + 1
    shas = {}
    for ver in ("v3", "v4"):
        shas[ver] = DveOpSpec(
            name="EXP8_CUBIC", opcode=row, uops=lower(spec, ver=ver), rd1_en=False
        ).sha(ver)
    op = dvo.DveOp("EXP8_CUBIC", spec, subdim=False, uops_sha=shas)
    dvo.OPS.append(op)
    dvo.CUSTOM_DVE_SPECS[op.name] = op.spec
    dvo._SUB_OPCODE_FOR_NAME[op.name] = row
    dvo.EXP8_CUBIC = op
    return op


def build_program(s=S, trace_sim=False):
    """Build the SPMD single-core program (parameterized seq len for sim)."""
    nch2 = s // CH2
    jb_n = s // 128
    nch1 = s // CH1
    exp_op = _get_exp_op() if USE_DVE_EXP else None

    nc = bacc.Bacc("TRN2", target_bir_lowering=False, debug=False, num_devices=8)

    xt = nc.dram_tensor("xt", [nch1, 128, KT, CH1], BF16, kind="ExternalInput")
    wqT = nc.dram_tensor("wqT", [128, KT, MG], BF16, kind="ExternalInput")
    wkT = nc.dram_tensor("wkT", [128, KT, MG], BF16, kind="ExternalInput")
    wvT = nc.dram_tensor("wvT", [128, KT, MG], BF16, kind="ExternalInput")
    woT = nc.dram_tensor("woT", [128, MT, D], BF16, kind="ExternalInput")
    trim = nc.dram_tensor("trim", [128, 128], BF16, kind="ExternalInput")
    vones = nc.dram_tensor("vones", [128, s // 128, NHL, 1], BF16,
                           kind="ExternalInput")
    y = nc.dram_tensor("y", [s, D], F32, kind="ExternalOutput")

    # greedy engine-balance accumulators (ns) for the attention phase;
    # DVE starts with a handicap for un-modeled queue overheads (measured
    # ~36us imbalance at equal modeled cost)
    eng = {"act": 0.0, "dve": 36000.0}

    def exp_cost_act(nfree):
        return (nfree + 352) / 1.2

    def exp_cost_dve(nfree):
        # measured: custom-DVE ~0.64ns/elem + ~250ns fixed (1224ns @ 1536)
        return nfree * 0.64 + 250.0

    with tile.TileContext(nc, trace_sim=trace_sim) as tc:
        with tc.tile_pool(name="persist", bufs=1) as pp:
            # ---- persistent SBUF ----
            qt = [pp.tile([128, s], BF16, tag=f"qt{m}", name=f"qt{m}")
                  for m in range(MT)]
            kt = [pp.tile([128, s], BF16, tag=f"kt{m}", name=f"kt{m}")
                  for m in range(MT)]
            # V with a ones column per head: [s-part, block, head, hd+1]
            vt = pp.tile([128, jb_n, NHL, HD + 1], BF16, tag="vt", name="vt")
            tri = pp.tile([128, 128], BF16, tag="tri", name="tri")
            # wo is needed only in phase 2 -- load it first on the sync ring,
            # which pays a ~15-19us warmup on its first large DMA
            wo_sb = [pp.tile([128, D], BF16, tag=f"wo{m}", name=f"wo{m}")
                     for m in range(MT)]
            for m in range(MT):
                nc.sync.dma_start(wo_sb[m][:], woT[:, m, :])

            nc.sync.dma_start(tri[:], trim[:])
            # ones column per (block, head) for the softmax denominator
            nc.sync.dma_start(vt[:, :, :, HD:HD + 1], vones[:])

            # =========== phase 1: projections (single x stream) ===========
            with (
                tc.tile_pool(name="ph1", bufs=2) as p1,
                tc.tile_pool(name="wsb", bufs=1) as pw,
                tc.tile_pool(name="ppsum", bufs=2, space="PSUM") as pps,
            ):
                wq_sb = pw.tile([128, KT, MG], BF16, tag="wq", name="wq_sb")
                wk_sb = pw.tile([128, KT, MG], BF16, tag="wk", name="wk_sb")
                wv_sb = pw.tile([128, KT, MG], BF16, tag="wv", name="wv_sb")
                xc0 = p1.tile([128, KT, CH1], BF16, tag="xc", name="xc")
                # phase-1-critical loads go on the ACT hwdge ring, which is
                # warm from the start (the sync ring's first big DMA pays the
                # warmup on the wo load above); k=0..1 slices first so the
                # first matmul group starts early
                nc.scalar.dma_start(wq_sb[:, 0:2, :], wqT[:, 0:2, :])
                nc.scalar.dma_start(xc0[:, 0:2, :], xt[0][:, 0:2, :])
                nc.scalar.dma_start(wq_sb[:, 2:, :], wqT[:, 2:, :])
                nc.scalar.dma_start(xc0[:, 2:, :], xt[0][:, 2:, :])
                nc.scalar.dma_start(wk_sb[:], wkT[:])
                nc.scalar.dma_start(wv_sb[:], wvT[:])

                for c in range(nch1):
                    xc = xc0 if c == 0 else p1.tile(
                        [128, KT, CH1], BF16, tag="xc", name="xc")
                    if c > 0:
                        nc.sync.dma_start(xc[:], xt[c])
                    cs = slice(c * CH1, (c + 1) * CH1)
                    for m in range(MT):
                        ms = slice(m * 128, (m + 1) * 128)
                        qp = pps.tile([128, CH1], F32, tag="p1q", name="qp")
                        kp = pps.tile([128, CH1], F32, tag="p1k", name="kp")
                        for k in range(KT):
                            nc.tensor.matmul(
                                qp[:], wq_sb[:, k, ms], xc[:, k, :],
                                start=(k == 0), stop=(k == KT - 1))
                        for k in range(KT):
                            nc.tensor.matmul(
                                kp[:], wk_sb[:, k, ms], xc[:, k, :],
                                start=(k == 0), stop=(k == KT - 1))
                        nc.scalar.copy(qt[m][:, cs], qp[:])
                        nc.scalar.copy(kt[m][:, cs], kp[:])
                    for st in range(CH1 // 128):
                        sb = c * (CH1 // 128) + st
                        vp = pps.tile([128, MG], F32, tag="p1v", name="vp")
                        for k in range(KT):
                            nc.tensor.matmul(
                                vp[:], xc[:, k, st * 128:(st + 1) * 128],
                                wv_sb[:, k, :],
                                start=(k == 0), stop=(k == KT - 1))
                        nc.vector.tensor_copy(
                            vt[:, sb, :, 0:HD],
                            vp[:].rearrange("p (h d) -> p h d", h=NHL))

            # =========== phase 2: attention + out-projection ===========
            with (
                tc.tile_pool(name="ph2", bufs=6) as p2,
                tc.tile_pool(name="pers2", bufs=1) as pp2,
                tc.tile_pool(name="ao", bufs=3) as pao,
                tc.tile_pool(name="nrm", bufs=4) as pn,
                tc.tile_pool(name="yst", bufs=2) as py,
                tc.tile_pool(name="spsum", bufs=5, space="PSUM") as sps,
                tc.tile_pool(name="pvps0", bufs=1, space="PSUM") as pv0p,
                tc.tile_pool(name="pvps1", bufs=1, space="PSUM") as pv1p,
                tc.tile_pool(name="ypsum", bufs=1, space="PSUM") as yps,
            ):
                ys_map = {}

                def outproj_group(cc, st, nn):
                    """One 512-wide psum group of the out-projection for
                    s-tile st of chunk cc.  Split so the groups can be
                    emitted interleaved into the next chunk's jb loop,
                    filling PE bubbles in the score->exp->PV chain."""
                    aocc = aoc_by_chunk[cc]
                    if nn == 0:
                        ys_map[(cc, st)] = py.tile([128, D], F32, tag="ys",
                                                   name="ys")
                    ys = ys_map[(cc, st)]
                    yp = yps.tile([128, 512], F32, tag="yp", name="yp")
                    for m in range(MT):
                        nc.tensor.matmul(
                            yp[:],
                            aocc[m][:, st * 128:(st + 1) * 128],
                            wo_sb[m][:, nn * 512:(nn + 1) * 512],
                            start=(m == 0), stop=(m == MT - 1))
                    # psum -> sbuf evacuation on the less-loaded engine
                    if eng["act"] + 720 <= eng["dve"] + 427:
                        eng["act"] += 720
                        nc.scalar.copy(ys[:, nn * 512:(nn + 1) * 512], yp[:])
                    else:
                        eng["dve"] += 427
                        nc.vector.tensor_copy(
                            ys[:, nn * 512:(nn + 1) * 512], yp[:])
                    if nn == D // 512 - 1:
                        r0 = cc * CH2 + st * 128
                        nc.sync.dma_start(y[r0:r0 + 128, :], ys[:])

                def outproj_stile(cc, st):
                    for nn in range(D // 512):
                        outproj_group(cc, st, nn)

                aoc_by_chunk = {}
                for c in range(nch2):
                    i0 = c * CH2
                    first_jb = 4 * c
                    # diag blocks mid-iteration: the leading full blocks
                    # give the exp engines a head start, the trailing ones
                    # let them catch up; psum start flag is on the first
                    # full-width matmul
                    full = list(range(first_jb + 4, jb_n))
                    diag = [first_jb + 3, first_jb + 2, first_jb + 1, first_jb]
                    mid = (len(full) + 1) // 2
                    jb_order = full[:mid] + diag + full[mid:]
                    # unnormalized attention output per chunk
                    aoc = [pao.tile([128, CH2], BF16, tag=f"ao{m}",
                                    name=f"ao{m}") for m in range(MT)]
                    aoc_by_chunk[c] = aoc
                    for hp in range(MT):  # head pair = m-tile
                        pvA = pv0p.tile([HD + 1, CH2], F32, tag="pv0",
                                        name="pv0")
                        pvB = pv1p.tile([HD + 1, CH2], F32, tag="pv1",
                                        name="pv1")
                        # previous chunk's out-projection groups, emitted
                        # interleaved into this jb loop as PE bubble filler
                        op_at = {}
                        if c > 0:
                            nj = len(jb_order)
                            op_at[max(nj // 3, 1)] = (c - 1, hp, 0)
                            op_at[max(2 * nj // 3, 2)] = (c - 1, hp, 1)
                        # software-pipeline: PV matmuls trail their scores by
                        # PVLAG iterations so the strict-FIFO PE queue never
                        # has a PV waiting on exp at its head
                        PVLAG = 2
                        pend = []

                        def emit_pv(ent):
                            jb2, n_r2, et2, st_f, sp_f = ent
                            nc.tensor.matmul(
                                pvA[:, 0:n_r2], vt[:, jb2, 2 * hp, :],
                                et2[:, 0:n_r2], start=st_f, stop=sp_f)
                            nc.tensor.matmul(
                                pvB[:, 0:n_r2], vt[:, jb2, 2 * hp + 1, :],
                                et2[:, CH2:CH2 + n_r2],
                                start=st_f, stop=sp_f)

                        for ji, jb in enumerate(jb_order):
                            if ji in op_at:
                                outproj_group(*op_at[ji])
                            rr = jb - first_jb
                            n_r = 128 * (rr + 1) if rr < 4 else CH2
                            ks = slice(jb * 128, (jb + 1) * 128)
                            # per-half single-bank score tiles from one
                            # 5-deep ring: deeper score->exp->free pipeline
                            scA = sps.tile([128, CH2], F32, tag="sc",
                                           name="scA")
                            scB = sps.tile([128, CH2], F32, tag="sc",
                                           name="scB")
                            nc.tensor.matmul(
                                scA[:, 0:n_r],
                                kt[hp][0:HD, ks], qt[hp][0:HD, i0:i0 + n_r],
                                start=True, stop=True)
                            nc.tensor.matmul(
                                scB[:, 0:n_r],
                                kt[hp][HD:128, ks], qt[hp][HD:128, i0:i0 + n_r],
                                start=True, stop=True)
                            # the two halves' exps go to different engines
                            # and run concurrently
                            et = p2.tile([128, 2 * CH2], BF16, tag="et",
                                         name="et")
                            for half, sct in ((0, scA), (1, scB)):
                                dst = et[:, half * CH2:half * CH2 + n_r]
                                ca = exp_cost_act(n_r)
                                cd = exp_cost_dve(n_r)
                                if exp_op is None or (eng["act"] + ca
                                                      <= eng["dve"] + cd):
                                    eng["act"] += ca
                                    nc.scalar.activation(
                                        dst, sct[:, 0:n_r], EXP,
                                        scale=ACT_SCALE)
                                else:
                                    eng["dve"] += cd
                                    nc.vector._custom_dve(
                                        exp_op, out=dst, in0=sct[:, 0:n_r],
                                        s0=_EXP_C0, s1=_EXP_C1, imm2=_EXP_C2)
                            if rr < 4:
                                dg = slice(128 * rr, n_r)
                                dgB = slice(CH2 + 128 * rr, CH2 + n_r)
                                eng["dve"] += 306
                                nc.vector.tensor_mul(et[:, dg], et[:, dg],
                                                     tri[:])
                                nc.vector.tensor_mul(et[:, dgB], et[:, dgB],
                                                     tri[:])
                            pend.append((jb, n_r, et,
                                         jb == jb_order[0],
                                         jb == jb_order[-1]))
                            if len(pend) > PVLAG:
                                emit_pv(pend.pop(0))
                        for ent in pend:
                            emit_pv(ent)
                        # normalize: approx-reciprocal of the ones-column
                        # denominators, partition-broadcast on GpSimd, one
                        # fused psum-read multiply per head
                        for half, pv in ((0, pvA), (1, pvB)):
                            # custom DVE ops require partition-0-based input
                            # APs: stage the denominator row first
                            den = pn.tile([1, CH2], F32, tag=f"den{half}",
                                          name=f"den{half}")
                            eng["act"] += 720
                            nc.scalar.copy(den[:], pv[HD:HD + 1, :])
                            rcp = pn.tile([1, CH2], F32, tag=f"rcp{half}",
                                          name=f"rcp{half}")
                            eng["dve"] += 690
                            nc.vector.reciprocal_approx_fast(rcp[:], den[:])
                            rcb = pn.tile([HD, CH2], F32, tag=f"rcb{half}",
                                          name=f"rcb{half}")
                            nc.gpsimd.partition_broadcast(rcb[:], rcp[:])
                            eng["dve"] += 427
                            nc.vector.tensor_mul(
                                aoc[hp][half * HD:(half + 1) * HD, :],
                                pv[0:HD, :], rcb[:])

                # last chunk's out-projection
                for st in range(CH2 // 128):
                    outproj_stile(nch2 - 1, st)

    nc.compile()
    return nc


_CACHE = {}


def _get_program():
    if "nc" not in _CACHE:
        _CACHE["nc"] = build_program()
    return _CACHE["nc"]


def _prep_inputs(x, wq, wk, wv, wo):
    """Per-core input maps. core = b + 4*g."""
    # triangular mask for the 128-col transition block of a diagonal key
    # block: keep (mul by 1) where key jj > query ii, else 0
    trim = np.where(
        np.arange(128)[:, None] > np.arange(128)[None, :], 1.0, 0.0
    ).astype(NPDT)
    vones = np.ones((128, S // 128, NHL, 1), dtype=NPDT)
    wqt = np.ascontiguousarray(wq.T)                      # (D, D): [d, m]
    wkt = np.ascontiguousarray(wk.T) * np.float32(WK_PRESCALE)
    wvt = np.ascontiguousarray(wv.T)
    wot = np.ascontiguousarray(wo.T)                      # [m, n]
    in_maps = []
    xts = {}
    for b in range(B):
        xT = x[b].T.astype(NPDT)  # (D, S)
        # [c, p, k, j] = xT[k*128+p, c*CH1+j]
        xts[b] = np.ascontiguousarray(
            xT.reshape(KT, 128, S // CH1, CH1).transpose(2, 1, 0, 3))

    def wslice(wt, ms):
        # [128, KT, MG] with [p, k, m] = wt[k*128+p, ms][m]
        return np.ascontiguousarray(
            wt[:, ms].reshape(KT, 128, MG).transpose(1, 0, 2).astype(NPDT))

    for core in range(8):
        b, g = core % 4, core // 4
        ms = slice(g * MG, (g + 1) * MG)
        # woT [128, MT, D]: [p, m, n] = wot[g*MG + m*128 + p, n]
        wo_core = np.ascontiguousarray(
            wot[ms, :].reshape(MT, 128, D).transpose(1, 0, 2).astype(NPDT))
        in_maps.append({
            "xt": xts[b],
            "wqT": wslice(wqt, ms),
            "wkT": wslice(wkt, ms),
            "wvT": wslice(wvt, ms),
            "woT": wo_core,
            "trim": trim,
            "vones": vones,
        })
    return in_maps


def _fix_last_rows(out, x, wq, wk, wv, wo, tail=64):
    """The last `tail` rows attend over few keys (no averaging to damp
    device bf16 noise), and row S-1 is fully masked (uniform softmax over
    all S keys).  Recompute them on host in fp64 -- cheap and exact."""
    q0 = S - tail
    wq64, wk64 = wq.astype(np.float64).T, wk.astype(np.float64).T
    wv64, wo64 = wv.astype(np.float64).T, wo.astype(np.float64).T
    hd = D // NH
    for b in range(B):
        xb = x[b].astype(np.float64)
        # row S-1: all keys masked -> uniform attention over all S keys
        vmean = xb.mean(axis=0) @ wv64
        out[b, S - 1, :] = (vmean @ wo64).astype(np.float32)
        # rows q0..S-2: keys strictly after the query, all within [q0+1, S)
        q6 = (xb[q0:S - 1] @ wq64).reshape(tail - 1, NH, hd)
        k6 = (xb[q0 + 1:] @ wk64).reshape(tail - 1, NH, hd)
        v6 = (xb[q0 + 1:] @ wv64).reshape(tail - 1, NH, hd)
        # scores[i, h, j] over keys global (q0+1+j); keep iff j >= i
        sc = np.einsum("ihd,jhd->hij", q6, k6) / np.sqrt(np.float64(D))
        keep = (np.arange(tail - 1)[None, :] >=
                np.arange(tail - 1)[:, None])[None, :, :]
        e = np.where(keep, np.exp(sc - sc.max(axis=2, keepdims=True)), 0.0)
        attn = e / e.sum(axis=2, keepdims=True)
        ao = np.einsum("hij,jhd->ihd", attn, v6).reshape(tail - 1, D)
        out[b, q0:S - 1, :] = (ao @ wo64).astype(np.float32)
    return out


def kernel(x, wq, wk, wv, wo, n_heads=NH, _trace=False):
    x = np.asarray(x, dtype=np.float32)
    wq = np.asarray(wq, dtype=np.float32)
    wk = np.asarray(wk, dtype=np.float32)
    wv = np.asarray(wv, dtype=np.float32)
    wo = np.asarray(wo, dtype=np.float32)

    nc = _get_program()
    in_maps = _prep_inputs(x, wq, wk, wv, wo)
    res = run_bass_kernel_spmd(nc, in_maps, list(range(8)), trace=_trace)
    out = np.zeros((B, S, D), dtype=np.float32)
    for b in range(B):
        out[b] = res.results[b]["y"] + res.results[b + 4]["y"]
    out = _fix_last_rows(out, x, wq, wk, wv, wo)
    if _trace:
        _CACHE["last_results"] = res
    return out
